# revision 1
# baseline (speedup 1.0000x reference)
"""Causal self-attention (GQA + qk RMS-norm + RoPE + q-gain) TRN2 Bass kernel.

Sharding: 8 cores = 2 batches x 4 kv-groups. Core c -> (b = c // 4, g = c % 4):
q heads 4g..4g+3, kv head g. Each core computes a partial projection output
(full [S, D]); host sums the 4 partials per batch.

Per-core program (SPMD, one BIR for all cores):
  inputs (f32r unless noted):
    xT    [2048, 2048]  x[b].T            (c-major: xT[c, t])
    wq    [2048, 512]   wq_g.T            ([c, o], o = 4 heads x 128)
    wkv   [2048, 256]   [wk_g.T | wv_g.T] ([c, 128+128])
    wproj [512, 2048]   wproj[:, 512g:512(g+1)].T  ([o, Dc])
    c2    [2048, 128]   f32  [cos | cos]  per (t, s, d2)
    s2    [2048, 128]   f32  [sin | -sin]
    gains [128, 4]      f32  gain[h]/sqrt(128), replicated over partitions
    maskw [128, 896]    staircase: maskw[j, u] = 1.0 if (u - 384) < j else 0
    negi  [128, 128]    -1e30 * I
    ident [128, 128]    I (for PE transposes, f32r)
    onescol [128, 1]    ones (denominator matmul lhsT)
    onesrow [1, 128]    f32 ones (broadcast matmul lhsT)
  output:
    out   [2048, 2048]  f32 partial projection output (natural [t, Dc])
"""
import sys

sys.path.insert(0, "/opt/trn_rl_repo")

from contextlib import ExitStack

import numpy as np

import concourse.bacc as bacc
import concourse.tile as tile
import concourse.mybir as mybir

F32 = mybir.dt.float32
F32R = mybir.dt.float32r
FP16 = mybir.dt.float16

S = 2048
D = 2048
HD = 128
NH_CORE = 4  # q heads per core
ROPE_BASE = 10000.0
EPS = 1.1920929e-07
NT = S // 128  # 16 t-tiles
NG = 4  # groups of 4 t-tiles
MASKVAL = -1e30
ALU = mybir.AluOpType
AF = mybir.ActivationFunctionType


def build_program(num_devices=8, phases="BCD", y16=True, p16=True):
    nc = bacc.Bacc("TRN2", target_bir_lowering=False, debug=False,
                   num_devices=num_devices)

    xT = nc.dram_tensor("xT", (D, S), F32R, kind="ExternalInput").ap()
    wq = nc.dram_tensor("wq", (D, 512), F32R, kind="ExternalInput").ap()
    wkv = nc.dram_tensor("wkv", (D, 256), F32R, kind="ExternalInput").ap()
    wproj = nc.dram_tensor("wproj", (512, D), FP16 if p16 else F32R, kind="ExternalInput").ap()
    c2d = nc.dram_tensor("c2", (S, 128), F32, kind="ExternalInput").ap()
    s2d = nc.dram_tensor("s2", (S, 128), F32, kind="ExternalInput").ap()
    gainsd = nc.dram_tensor("gains", (128, 4), F32, kind="ExternalInput").ap()
    keepwd = nc.dram_tensor("keepw", (128, 896), FP16, kind="ExternalInput").ap()
    identd = nc.dram_tensor("ident", (128, 128), F32R, kind="ExternalInput").ap()
    onescd = nc.dram_tensor("onescol", (128, 1), FP16 if y16 else F32R, kind="ExternalInput").ap()
    onesrd = nc.dram_tensor("onesrow", (1, 128), F32, kind="ExternalInput").ap()
    outd = nc.dram_tensor("out", (S, D), F32, kind="ExternalOutput").ap()

    with tile.TileContext(nc) as tc, ExitStack() as ctx:
        # ---------------- persistent pools ----------------
        sbc = ctx.enter_context(tc.tile_pool(name="consts", bufs=1))
        qtp = ctx.enter_context(tc.tile_pool(name="qtp", bufs=16))

        # ---------------- constants / weights resident ----------------
        wq_sb = sbc.tile([128, 16 * 512], F32R, tag="wq")
        wkv_sb = sbc.tile([128, 16 * 256], F32R, tag="wkv")
        for k in range(0, 16, 2):
            nc.gpsimd.dma_start(
                wq_sb[:, k * 512:(k + 2) * 512].rearrange(
                    "p (k o) -> p k o", k=2),
                wq[k * 128:(k + 2) * 128, :].rearrange(
                    "(k p) o -> p k o", p=128),
            )
            nc.gpsimd.dma_start(
                wkv_sb[:, k * 256:(k + 2) * 256].rearrange(
                    "p (k o) -> p k o", k=2),
                wkv[k * 128:(k + 2) * 128, :].rearrange(
                    "(k p) o -> p k o", p=128),
            )
        c2_sb = sbc.tile([128, 16 * 128], F32, tag="c2")
        nc.gpsimd.dma_start(
            c2_sb[:].rearrange("p (i d) -> p i d", i=16),
            c2d.rearrange("(i p) d -> p i d", p=128),
        )
        s2_sb = sbc.tile([128, 16 * 128], F32, tag="s2")
        nc.gpsimd.dma_start(
            s2_sb[:].rearrange("p (i d) -> p i d", i=16),
            s2d.rearrange("(i p) d -> p i d", p=128),
        )
        gains_sb = sbc.tile([128, 4], F32, tag="gains")
        nc.gpsimd.dma_start(gains_sb[:], gainsd)
        keepw_sb = sbc.tile([128, 896], FP16, tag="keepw")
        nc.gpsimd.dma_start(keepw_sb[:], keepwd)
        ident_sb = sbc.tile([128, 128], F32R, tag="ident")
        nc.gpsimd.dma_start(ident_sb[:], identd)
        onesc_sb = sbc.tile([128, 1], FP16 if y16 else F32R, tag="onesc")
        nc.gpsimd.dma_start(onesc_sb[:], onescd)
        onesr_sb = sbc.tile([1, 128], F32, tag="onesr")
        nc.gpsimd.dma_start(onesr_sb[:], onesrd)

        negb_sb = sbc.tile([128, 1], F32, tag="negb")
        nc.vector.memset(negb_sb[:], -1.0)
        kT_sb = sbc.tile([128, 16 * 128], F32R, tag="kT")     # k final, transposed
        v_sb = sbc.tile([128, 16 * 128], FP16 if y16 else F32R, tag="v")   # v natural

        qT = {}   # (h, g) -> [128 d, 512 t] tile
        ytile = {}  # (h, qc) -> [128 d, 512 t] tile

        # ================ phase B: projections + norm + rope + transpose ====
        ctxB = ExitStack()
        io2k = ctxB.enter_context(tc.tile_pool(name="io2k", bufs=17))
        work = ctxB.enter_context(tc.tile_pool(name="work", bufs=1))
        qfp = ctxB.enter_context(tc.tile_pool(name="qfp", bufs=1))
        smp = ctxB.enter_context(tc.tile_pool(name="smp", bufs=4))
        psQ = ctxB.enter_context(tc.tile_pool(name="psQ", bufs=2, space="PSUM"))
        psKV = ctxB.enter_context(tc.tile_pool(name="psKV", bufs=2, space="PSUM"))
        psTR = ctxB.enter_context(tc.tile_pool(name="psTR", bufs=2, space="PSUM"))

        for g in range(NG):
            xts = []
            for k in range(16):
                xt = io2k.tile([128, 512], F32R, tag="io", name=f"xt_{g}_{k}")
                nc.sync.dma_start(
                    xt[:], xT[k * 128:(k + 1) * 128, g * 512:(g + 1) * 512]
                )
                xts.append(xt)
            xts = [t[:] for t in xts]

            qn = work.tile([128, 2048], F32, tag="qn", name=f"qn_{g}")
            kn = work.tile([128, 512], F32, tag="kn", name=f"kn_{g}", bufs=2)
            for tt in range(4):
                i = g * 4 + tt
                psq = psQ.tile([128, 512], F32, tag="pQ", name=f"psq_{i}")
                for k in range(16):
                    nc.tensor.matmul(
                        psq[:],
                        xts[k][:, tt * 128:(tt + 1) * 128],
                        wq_sb[:, k * 512:(k + 1) * 512],
                        start=(k == 0), stop=(k == 15),
                    )
                pskv = psKV.tile([128, 256], F32, tag="pKV", name=f"pskv_{i}")
                for k in range(16):
                    nc.tensor.matmul(
                        pskv[:],
                        xts[k][:, tt * 128:(tt + 1) * 128],
                        wkv_sb[:, k * 256:(k + 1) * 256],
                        start=(k == 0), stop=(k == 15),
                    )

                # evacuate raw, stats on DVE (keeps ACT on the ln/exp set)
                qraw = smp.tile([128, 512], F32, tag="qraw",
                                name=f"qraw_{i}", bufs=2)
                nc.vector.tensor_copy(qraw[:], psq[:])
                kvraw = smp.tile([128, 256], F32, tag="kvraw",
                                 name=f"kvraw_{i}", bufs=2)
                nc.vector.tensor_copy(kvraw[:], pskv[:])
                ms = smp.tile([128, 5], F32, tag="ms", name=f"ms_{i}")
                for h in range(NH_CORE):
                    nc.vector.scalar_tensor_tensor(
                        out=qn[:, tt * 512 + h * 128: tt * 512 + (h + 1) * 128],
                        in0=qraw[:, h * 128:(h + 1) * 128],
                        scalar=1.0,
                        in1=qraw[:, h * 128:(h + 1) * 128],
                        op0=ALU.mult, op1=ALU.mult,
                        accum_out=ms[:, h:h + 1],
                    )
                nc.vector.scalar_tensor_tensor(
                    out=kn[:, tt * 128:(tt + 1) * 128],
                    in0=kvraw[:, 0:128], scalar=1.0, in1=kvraw[:, 0:128],
                    op0=ALU.mult, op1=ALU.mult,
                    accum_out=ms[:, 4:5],
                )
                msx = smp.tile([128, 5], F32, tag="msx", name=f"msx_{i}")
                nc.vector.tensor_scalar(msx[:], ms[:], 1.0 / HD, EPS,
                                        op0=ALU.mult, op1=ALU.add)
                u = smp.tile([128, 5], F32, tag="u", name=f"u_{i}")
                usc = smp.tile([128, 5], F32, tag="usc", name=f"usc_{i}")
                nc.vector.reciprocal_approx_accurate(out=u[:], in_=msx[:],
                                                     scratch=usc[:])
                rin = smp.tile([128, 5], F32, tag="rin", name=f"rin_{i}")
                nc.scalar.activation(rin[:], u[:], AF.Sqrt)
                ring = smp.tile([128, 4], F32, tag="ring", name=f"ring_{i}")
                nc.vector.tensor_mul(ring[:], rin[:, 0:4], gains_sb[:])

                # scale into qn / kn, copy v
                for h in range(NH_CORE):
                    nc.vector.tensor_scalar_mul(
                        qn[:, tt * 512 + h * 128: tt * 512 + (h + 1) * 128],
                        qraw[:, h * 128:(h + 1) * 128],
                        ring[:, h:h + 1],
                    )
                nc.vector.tensor_scalar_mul(
                    kn[:, tt * 128:(tt + 1) * 128],
                    kvraw[:, 0:128], rin[:, 4:5],
                )
                nc.scalar.copy(
                    v_sb[:, i * 128:(i + 1) * 128], kvraw[:, 128:256],
                )

            # ---- rope on q group: [tt, h, s, d2] layout ----
            qn5 = qn[:].rearrange("p (tt h s d) -> p tt h s d", tt=4, h=4, s=2)
            c2g = (
                c2_sb[:].rearrange("p (i one s d) -> p i one s d",
                                   i=16, one=1, s=2)
                [:, g * 4:(g + 1) * 4]
                .broadcast_to((128, 4, 4, 2, 64))
            )
            s2g = (
                s2_sb[:].rearrange("p (i one s d) -> p i one s d",
                                   i=16, one=1, s=2)
                [:, g * 4:(g + 1) * 4]
                .broadcast_to((128, 4, 4, 2, 64))
            )
            t1 = work.tile([128, 2048], F32, tag="rt", name=f"t1_{g}")
            qf = qfp.tile([128, 2048], F32R, tag="qf", name=f"qf_{g}")
            t15 = t1[:].rearrange("p (tt h s d) -> p tt h s d", tt=4, h=4, s=2)
            qf5 = qf[:].rearrange("p (tt h s d) -> p tt h s d", tt=4, h=4, s=2)
            nc.vector.tensor_mul(t15[:, :, :, 0:1, :], qn5[:, :, :, 1:2, :],
                                 s2g[:, :, :, 0:1, :])
            nc.vector.tensor_mul(t15[:, :, :, 1:2, :], qn5[:, :, :, 0:1, :],
                                 s2g[:, :, :, 1:2, :])
            nc.vector.tensor_mul(qf5, qn5, c2g)
            nc.vector.tensor_add(qf[:], qf[:], t1[:])

            # ---- rope on k group: [i(4), s, d2] layout ----
            kn4 = kn[:].rearrange("p (i s d) -> p i s d", i=4, s=2)
            kc2 = c2_sb[:, g * 512:(g + 1) * 512].rearrange(
                "p (i s d) -> p i s d", i=4, s=2)
            ks2 = s2_sb[:, g * 512:(g + 1) * 512].rearrange(
                "p (i s d) -> p i s d", i=4, s=2)
            kt1 = work.tile([128, 512], F32, tag="kt", name=f"kt1_{g}", bufs=2)
            kf = work.tile([128, 512], F32R, tag="kf", name=f"kf_{g}", bufs=2)
            kt14 = kt1[:].rearrange("p (i s d) -> p i s d", i=4, s=2)
            kf4 = kf[:].rearrange("p (i s d) -> p i s d", i=4, s=2)
            nc.vector.tensor_mul(kt14[:, :, 0:1, :], kn4[:, :, 1:2, :],
                                 ks2[:, :, 0:1, :])
            nc.vector.tensor_mul(kt14[:, :, 1:2, :], kn4[:, :, 0:1, :],
                                 ks2[:, :, 1:2, :])
            nc.vector.tensor_mul(kf4, kn4, kc2)
            nc.vector.tensor_add(kf[:], kf[:], kt1[:])

            # ---- transposes: q (h, g) -> qT, k -> kT_sb ----
            for h in range(NH_CORE):
                trp = psTR.tile([128, 512], F32R, tag="pTR", name=f"trq_{g}_{h}")
                for tt in range(4):
                    nc.tensor.transpose(
                        trp[:, tt * 128:(tt + 1) * 128],
                        qf[:, tt * 512 + h * 128: tt * 512 + (h + 1) * 128],
                        ident_sb[:],
                    )
                qt = qtp.tile([128, 512], F32R, tag="qT", name=f"qT_{g}_{h}")
                nc.scalar.copy(qt[:], trp[:])
                qT[(h, g)] = qt
            trk = psTR.tile([128, 512], F32R, tag="pTR", name=f"trk_{g}")
            for tt in range(4):
                nc.tensor.transpose(
                    trk[:, tt * 128:(tt + 1) * 128],
                    kf[:, tt * 128:(tt + 1) * 128],
                    ident_sb[:],
                )
            nc.scalar.copy(kT_sb[:, g * 512:(g + 1) * 512], trk[:])
        ctxB.close()

        # ================ phase C: attention =================
        ctxC = ExitStack()
        expp = ctxC.enter_context(tc.tile_pool(name="expp", bufs=3))
        smc = ctxC.enter_context(tc.tile_pool(name="smc", bufs=2))
        ytp = ctxC.enter_context(tc.tile_pool(name="ytp", bufs=8))
        wpp = ctxC.enter_context(tc.tile_pool(name="wpp", bufs=16))
        outp = ctxC.enter_context(tc.tile_pool(name="outp", bufs=2))
        psSC = ctxC.enter_context(tc.tile_pool(name="psSC", bufs=2, space="PSUM"))
        psYT = ctxC.enter_context(tc.tile_pool(name="psYT", bufs=2, space="PSUM"))
        psDB = ctxC.enter_context(tc.tile_pool(name="psDB", bufs=2, space="PSUM"))
        psFP = ctxC.enter_context(tc.tile_pool(name="psFP", bufs=2, space="PSUM"))

        for qc in range(4 if "C" in phases else 0):
            jmax = 4 * qc + 3
            for h in range(NH_CORE):
                yt_ps = psYT.tile([128, 512], F32, tag="pYT", name=f"yt_{qc}_{h}")
                den_ps = psDB.tile([1, 512], F32, tag="pDB",
                                    name=f"den_{qc}_{h}")
                for j in range(jmax + 1):
                    diag = j >= 4 * qc
                    sc = psSC.tile([128, 512], F32, tag="pSC",
                                   name=f"sc_{qc}_{h}_{j}")
                    nc.tensor.matmul(
                        sc[:],
                        kT_sb[:, j * 128:(j + 1) * 128],
                        qT[(h, qc)][:],
                        start=True, stop=True,
                    )
                    ex = expp.tile([128, 512], FP16 if y16 else F32R,
                                   tag="exp", name=f"ex_{qc}_{h}_{j}")
                    nc.scalar.activation(ex[:], sc[:], AF.Exp,
                                         bias=negb_sb[:] if y16 else 0.0)
                    if diag:
                        dlt = 128 * j - 512 * qc
                        nc.vector.tensor_mul(
                            ex[:], ex[:],
                            keepw_sb[:, 384 - dlt: 896 - dlt],
                        )
                    nc.tensor.matmul(den_ps[:], onesc_sb[:], ex[:],
                                     start=(j == 0), stop=(j == jmax))
                    nc.tensor.matmul(yt_ps[:],
                                     v_sb[:, j * 128:(j + 1) * 128],
                                     ex[:], start=(j == 0), stop=(j == jmax))
                rinv = smc.tile([1, 512], F32, tag="rinv",
                                name=f"rinv_{qc}_{h}")
                nc.vector.reciprocal_approx_fast(out=rinv[:], in_=den_ps[:])
                bc_ps = psDB.tile([128, 512], F32, tag="pDB",
                                  name=f"bc_{qc}_{h}")
                nc.tensor.matmul(bc_ps[:], onesr_sb[:], rinv[:],
                                 start=True, stop=True)
                bc_sb = smc.tile([128, 512], F32, tag="bcs",
                                 name=f"bcs_{qc}_{h}")
                nc.vector.tensor_copy(bc_sb[:], bc_ps[:])
                yt = ytp.tile([128, 512], FP16 if p16 else F32R, tag="yt",
                              name=f"ytsb_{qc}_{h}")
                nc.vector.tensor_mul(yt[:], yt_ps[:], bc_sb[:])
                ytile[(h, qc)] = yt

        # ================ phase D: output projection (interleaved) =======
        if "D" in phases:
            wp = {}
            for h in range(NH_CORE):
                for dc in range(4):
                    w = wpp.tile([128, 512], FP16 if p16 else F32R,
                                 tag="wp", name=f"wp_{h}_{dc}")
                    nc.gpsimd.dma_start(
                        w[:],
                        wproj[h * 128:(h + 1) * 128, dc * 512:(dc + 1) * 512],
                    )
                    wp[(h, dc)] = w
            for qc in range(4):
                for tt in range(4):
                    i = qc * 4 + tt
                    for dc2 in range(2):
                        ob = outp.tile([128, 1024], F32, tag="ob",
                                       name=f"ob_{i}_{dc2}")
                        for half in range(2):
                            dc = dc2 * 2 + half
                            fp = psFP.tile([128, 512], F32, tag="pFP",
                                           name=f"fp_{i}_{dc}")
                            for h in range(NH_CORE):
                                nc.tensor.matmul(
                                    fp[:],
                                    ytile[(h, qc)][:, tt * 128:(tt + 1) * 128],
                                    wp[(h, dc)][:],
                                    start=(h == 0), stop=(h == 3),
                                )
                            if (i + dc) % 2 == 0:
                                nc.vector.tensor_copy(
                                    ob[:, half * 512:(half + 1) * 512], fp[:])
                            else:
                                nc.scalar.copy(
                                    ob[:, half * 512:(half + 1) * 512], fp[:])
                        nc.gpsimd.dma_start(
                            outd[i * 128:(i + 1) * 128,
                                 dc2 * 1024:(dc2 + 1) * 1024],
                            ob[:],
                        )
        ctxC.close()

    nc.compile()
    return nc


# ---------------- host-side helpers ----------------

def rope_tables():
    inv_freq = 1.0 / (ROPE_BASE ** (np.arange(0, HD, 2, dtype=np.float32) / HD))
    t = np.arange(S, dtype=np.float32)
    fr = np.outer(t, inv_freq)
    cos = np.cos(fr).astype(np.float32)
    sin = np.sin(fr).astype(np.float32)
    c2 = np.concatenate([cos, cos], axis=1)
    s2 = np.concatenate([sin, -sin], axis=1)
    return c2, s2


def make_consts():
    c2, s2 = rope_tables()
    j = np.arange(128)[:, None]
    u = np.arange(896)[None, :]
    keepw = ((u - 384) >= j).astype(np.float16)
    ident = np.eye(128, dtype=np.float32)
    onescol = np.ones((128, 1), np.float16)
    onesrow = np.ones((1, 128), np.float32)
    return dict(c2=c2, s2=s2, keepw=keepw, ident=ident,
                onescol=onescol, onesrow=onesrow)


def make_core_inputs(x, wq, wk, wv, wproj, q_gain, core, consts=None):
    """x: [B, S, D] f32; returns in_map for `core` (0..7)."""
    if consts is None:
        consts = make_consts()
    b, g = core // 4, core % 4
    xTc = np.ascontiguousarray(x[b].T)                       # [D, S]
    wqc = np.ascontiguousarray(wq[g * 512:(g + 1) * 512].T)  # [D, 512]
    wkc = wk[g * 128:(g + 1) * 128].T                        # [D, 128]
    wvc = wv[g * 128:(g + 1) * 128].T
    wkvc = np.ascontiguousarray(np.concatenate([wkc, wvc], axis=1))
    wpc = np.ascontiguousarray(
        wproj[:, g * 512:(g + 1) * 512].T.astype(np.float16))  # [512, D]
    gains = np.broadcast_to(
        (q_gain[g * 4:(g + 1) * 4] / np.sqrt(HD)).astype(np.float32)[None, :],
        (128, 4),
    ).copy()
    return dict(
        xT=xTc, wq=wqc, wkv=wkvc, wproj=wpc,
        c2=consts["c2"], s2=consts["s2"], gains=gains,
        keepw=consts["keepw"], ident=consts["ident"],
        onescol=consts["onescol"], onesrow=consts["onesrow"],
    )


# ---------------- public entry point ----------------

_PROGRAM = None


def _get_program():
    global _PROGRAM
    if _PROGRAM is None:
        _PROGRAM = build_program()
    return _PROGRAM


def kernel(x, wq, wk, wv, wproj, q_gain):
    """Causal self-attention forward. Full inputs in, full output out.

    Shards across 8 NeuronCores as 2 batches x 4 kv-head groups
    (tensor-parallel over heads); each core produces a partial output
    projection; partials are summed per batch on the host (the unshard
    step for input-dim-sharded wproj).
    """
    from concourse.bass_utils import run_bass_kernel_spmd

    x = np.ascontiguousarray(np.asarray(x, dtype=np.float32))
    wq = np.ascontiguousarray(np.asarray(wq, dtype=np.float32))
    wk = np.ascontiguousarray(np.asarray(wk, dtype=np.float32))
    wv = np.ascontiguousarray(np.asarray(wv, dtype=np.float32))
    wproj = np.ascontiguousarray(np.asarray(wproj, dtype=np.float32))
    q_gain = np.asarray(q_gain, dtype=np.float32)

    nc = _get_program()
    consts = make_consts()
    in_maps = [make_core_inputs(x, wq, wk, wv, wproj, q_gain, c, consts)
               for c in range(8)]
    res = run_bass_kernel_spmd(nc, in_maps, core_ids=list(range(8)))
    parts = [r["out"].astype(np.float32) for r in res.results]
    y = np.stack([
        parts[0] + parts[1] + parts[2] + parts[3],
        parts[4] + parts[5] + parts[6] + parts[7],
    ]).astype(np.float32)
    return y

